# revision 16
# baseline (speedup 1.0000x reference)
"""Trainium2 Bass kernel for nn_Blurring_Model: 3D Gaussian blur (9^3 PSF)
on x[8,1,128,128,128] fp32, data-parallel over batch across 8 NeuronCores.

Method (per core, volume V[128,128,128]):
  The 3D PSF is separable: three 9-tap 1D convolutions along D, H, W.
  Each 1D conv along the SBUF partition axis is a matmul with a banded
  128x128 matrix B (B[d, d'] = g[d - d' + 4]); zero band entries handle
  the 'same' zero padding exactly.

  Every pass uses the volume chunk as the matmul's stationary operand
  (lhsT) and streams B, so out = chunk^T @ B both convolves the current
  partition axis AND rotates the next axis onto partitions ("rotation
  matmul") - no explicit transposes:

    V0 [D,(Hmaj,Wmin)] --pass1 conv D (chunks: fixed w)--> V1 [H,(Dmaj,Wmin)]
    V1                 --pass2 conv H (chunks: fixed d)--> V2 [W,(Hmaj,Dmin)]
    V2                 --pass3 conv W (chunks: fixed h)--> V3 [D,(Hmaj,Wmin)]

  128 matmuls of [K=128, M=128] x [K=128, N] per pass.  In "f32r" mode the
  streamed B is padded to N=256 so the PE runs float32r at 1 cycle/row.
  PSUM->SBUF copies are batched 8 chunks at a time and alternate between
  the Vector and Scalar engines.
"""

import sys

if "/opt/trn_rl_repo" not in sys.path:
    sys.path.insert(0, "/opt/trn_rl_repo")

import numpy as np

KERNEL_SIZE = 9
SPACING = (4.0, 4.0, 4.0)
CENTER = (KERNEL_SIZE - 1) / 2.0
PAD = (KERNEL_SIZE - 1) // 2
P = 128
HW = P * P
N_CORES = 8

# "f32r" | "f32" | "fp16" | "bf16"
MODE = "f32r"

GRP = 8           # matmul chunks per PSUM group / copy
NGRP = P // GRP   # groups per pass

_cache = {}


def _gauss1d(sigma, spacing):
    s = float(sigma) / spacing
    xs = np.arange(KERNEL_SIZE, dtype=np.float64)
    g = np.exp(-((xs - CENTER) ** 2) / (2.0 * s * s))
    g = g / g.sum()
    return g.astype(np.float32)


def _banded(g, ncols):
    # B[d, d'] = g[d - d' + PAD] for |d - d'| <= PAD, else 0.
    B = np.zeros((P, ncols), dtype=np.float32)
    d = np.arange(P)
    for i in range(KERNEL_SIZE):
        off = i - PAD
        dp = d - off
        m = (dp >= 0) & (dp < P)
        B[d[m], dp[m]] = g[i]
    return B


def _mode_params(mode):
    import concourse.mybir as mybir

    f32 = mybir.dt.float32
    if mode == "fp16":
        return mybir.dt.float16, np.float16, 128, 96
    if mode == "bf16":
        import ml_dtypes

        return mybir.dt.bfloat16, np.dtype(ml_dtypes.bfloat16), 128, 96
    if mode == "f32":
        return f32, np.float32, 128, 96
    if mode == "f32r":
        return f32, np.float32, 256, 120
    raise ValueError(mode)


def _build(mode):
    """Builds the SPMD Bass module (single program, run on 8 cores)."""
    if mode in _cache:
        return _cache[mode]

    from contextlib import ExitStack

    import concourse.bacc as bacc
    import concourse.bass as bass
    import concourse.mybir as mybir
    import concourse.tile as tile

    f32 = mybir.dt.float32
    dt_dat, _, NB, n_warm = _mode_params(mode)
    f32r_mode = mode == "f32r"
    # dtype the matmuls consume (and the volume tiles are stored as)
    dt_vol = mybir.dt.float32r if f32r_mode else dt_dat

    nc = bacc.Bacc(trn_type="TRN2", target_bir_lowering=False, debug=False)
    x_in = nc.declare_dram_parameter(
        "x", [P, HW], dt_vol if f32r_mode else dt_dat, isOutput=False
    ).ap()
    b_in = nc.declare_dram_parameter("bmats", [P, 3 * NB], dt_dat, isOutput=False).ap()
    y_out = nc.declare_dram_parameter("y", [P, HW], f32, isOutput=True).ap()

    with ExitStack() as ctx:
        tc = ctx.enter_context(tile.TileContext(nc))
        vol = ctx.enter_context(tc.tile_pool(name="vol", bufs=3))
        consts = ctx.enter_context(tc.tile_pool(name="consts", bufs=1))
        pspool = ctx.enter_context(tc.tile_pool(name="ps", bufs=2, space="PSUM"))

        braw = consts.tile([P, 3 * NB], dt_dat, name="braw", tag="braw")
        nc.sync.dma_start(out=braw[:], in_=b_in[:])
        btile = consts.tile([P, 3 * NB], dt_vol, name="btile", tag="b")
        # engine copy rounds f32 -> f32r as the BIR verifier requires
        nc.vector.tensor_copy(out=btile[:], in_=braw[:])
        scratch = consts.tile([P, 128], f32, name="scratch", tag="scratch")

        v0 = vol.tile([P, HW], dt_vol, name="v0", tag="vol")
        nc.sync.dma_start(out=v0[:], in_=x_in[:])

        # Two persistent PSUM tiles, ping-ponged by group parity.  Never
        # recycling tiles keeps the PE->PE PSUM WAW deps semaphore-free
        # (program order), so each matmul carries at most ONE sync wait -
        # the LDWEIGHTS instruction encoding cannot hold more.
        psA = pspool.tile([P, GRP * NB], f32, name="psA", tag="ps")
        psB = pspool.tile([P, GRP * NB], f32, name="psB", tag="ps")

        # Warm the ACT tables (Copy) and the PE HAM clock gate while the
        # 8MB input DMA is in flight.
        nc.scalar.copy(out=scratch[:], in_=braw[:, 0:128])
        for _ in range(n_warm):
            nc.tensor.matmul(
                out=psA[:, 0:NB],
                lhsT=btile[:, 0:128],
                rhs=btile[:, 0:NB],
                start=True,
                stop=True,
            )

        v1 = vol.tile([P, HW], dt_vol, name="v1", tag="vol")
        v2 = vol.tile([P, HW], dt_vol, name="v2", tag="vol")
        v3 = vol.tile([P, HW], f32, name="v3", tag="vol")

        def conv_pass(src, dst, b_idx, chunk_fn, dst_ap_fn, pass_idx):
            b_ap = btile[:, b_idx * NB : (b_idx + 1) * NB]
            dve_copies = []
            for g in range(NGRP):
                ps = psA if g % 2 == 0 else psB
                for c in range(GRP):
                    nc.tensor.matmul(
                        out=ps[:, c * NB : (c + 1) * NB],
                        lhsT=chunk_fn(src, g * GRP + c),
                        rhs=b_ap,
                        start=True,
                        stop=True,
                    )
                src_ap, dst_ap = dst_ap_fn(ps, dst, g)
                if g % 2 == 0:
                    cp = nc.vector.tensor_copy(out=dst_ap, in_=src_ap)
                    dve_copies.append(cp)
                else:
                    nc.scalar.copy(out=dst_ap, in_=src_ap)
                if pass_idx == 2 and g % 2 == 1:
                    j = g // 2
                    nc.sync.dma_start(
                        out=y_out[:, j * 2048 : (j + 1) * 2048],
                        in_=v3[:, j * 2048 : (j + 1) * 2048],
                    )
            return dve_copies

        # pass 1: conv D.  src free = h*128 + w; chunk = fixed w -> [d, h]
        # out [h, d'] -> V1[h, d'*128 + w]
        def chunk1(src, w):
            return src.rearrange("p (h w) -> p w h", w=P)[:, w, :]

        def dst1(ps, dst, g):
            src_ap = ps.rearrange("p (c n) -> p n c", n=NB)[:, 0:128, :]
            dst_ap = dst.rearrange("p (dp w) -> p dp w", w=P)[:, :, g * GRP : (g + 1) * GRP]
            return src_ap, dst_ap

        # pass 2: conv H.  V1 free = d*128 + w; chunk = fixed d -> [h, w]
        # out [w, h'] -> V2[w, h'*128 + d]
        def chunk2(src, d):
            return src[:, d * P : (d + 1) * P]

        def dst2(ps, dst, g):
            src_ap = ps.rearrange("p (c n) -> p n c", n=NB)[:, 0:128, :]
            dst_ap = dst.rearrange("p (hp d) -> p hp d", d=P)[:, :, g * GRP : (g + 1) * GRP]
            return src_ap, dst_ap

        # pass 3: conv W.  V2 free = h*128 + d; chunk = fixed h -> [w, d]
        # out [d, w'] -> V3[d, h*128 + w']
        def chunk3(src, h):
            return src[:, h * P : (h + 1) * P]

        def dst3(ps, dst, g):
            src_ap = ps.rearrange("p (c n) -> p c n", n=NB)[:, :, 0:128]
            dst_ap = dst.rearrange("p (h wp) -> p h wp", wp=P)[:, g * GRP : (g + 1) * GRP, :]
            return src_ap, dst_ap

        from concourse.tile_rust import add_dep_helper

        def pass_boundary(dve_copies, idx):
            # The first matmul of the next pass depends on all 16 copies of
            # the previous pass (true all-to-all), which would give its
            # LDWEIGHTS 2+ sync waits - the encoding holds only one.  Wait
            # dedup in Tile only works matmul-to-matmul, so interpose a tiny
            # dummy matmul (M=1, N=1) that carries the DVE-side wait; the
            # first real matmul then only needs the ACT-side wait.
            mmi = nc.tensor.matmul(
                out=psA[0:32, 0:NB],
                lhsT=btile[:, 0:32],
                rhs=btile[:, 0:NB],
                start=True,
                stop=True,
            )
            for cp in dve_copies:
                add_dep_helper(
                    mmi.ins, cp.ins, sync=True, reason="pass boundary wait split"
                )

        d1 = conv_pass(v0, v1, 0, chunk1, dst1, 0)
        pass_boundary(d1, 0)
        d2 = conv_pass(v1, v2, 1, chunk2, dst2, 1)
        pass_boundary(d2, 1)
        conv_pass(v2, v3, 2, chunk3, dst3, 2)

    nc.compile()
    _cache[mode] = nc
    return nc


def _prep_inputs(x, sigma_x, sigma_y, sigma_z, mode):
    _, np_dt, NB, _ = _mode_params(mode)
    gx = _gauss1d(float(sigma_x), SPACING[0])
    gy = _gauss1d(float(sigma_y), SPACING[1])
    gz = _gauss1d(float(sigma_z), SPACING[2])
    bmats = np.concatenate(
        [_banded(gx, NB), _banded(gy, NB), _banded(gz, NB)], axis=1
    ).astype(np_dt)
    x = np.asarray(x, dtype=np.float32).reshape(N_CORES, P, HW)
    in_maps = [
        {"x": np.ascontiguousarray(x[i]).astype(np_dt), "bmats": bmats}
        for i in range(N_CORES)
    ]
    return in_maps


def _run(x, sigma_x, sigma_y, sigma_z, mode=None, trace=False):
    from concourse.bass_utils import run_bass_kernel_spmd

    mode = mode or MODE
    nc = _build(mode)
    in_maps = _prep_inputs(x, sigma_x, sigma_y, sigma_z, mode)
    res = run_bass_kernel_spmd(nc, in_maps, core_ids=list(range(N_CORES)), trace=trace)
    y = np.stack([np.asarray(res.results[i]["y"]) for i in range(N_CORES)])
    y = y.reshape(N_CORES, 1, P, P, P).astype(np.float32)
    return y, res


def kernel(x, sigma_x, sigma_y, sigma_z):
    y, _ = _run(x, sigma_x, sigma_y, sigma_z)
    return y
